# revision 19
# baseline (speedup 1.0000x reference)
"""Trainium2 Bass kernel for the CoAttention DNS/Image module.

Math notes (exact algebraic simplification of the reference):
  scores1[b,r,s] = s_img[b,r] + s_dns[b,s] + b_att1 ; softmax over s.
  The per-row constant s_img[b,r] (and b_att1) cancels in the softmax, so
  a1[b,r,:] == softmax(s_dns[b,:]) for every r. Hence
      att_dns[b,r,:] = softmax(s_dns[b]) @ dns[b]          (same for all r)
  Similarly scores2's softmax over j kills t_dns[b,i] and b_att2, so
      att_img[b,i,:] = softmax(t_img[b]) @ img[b]          (same for all i)
  Therefore W_img1, w_att1[:H], b_att1, W_dns2, w_att2[:H], b_att2 do not
  affect the outputs at all.  The remaining work per batch item:
      s_dns[s] = tanh(dns[b] @ W_dns1.T + b_dns1) @ w_att1[H:]
      t_img[j] = tanh(img[b] @ W_img2.T + b_img2) @ w_att2[H:]
  plus two softmaxes and two weighted sums.  Because every output row of
  att_dns[b] (and att_img[b]) is identical, the device only produces one
  row per item; the host broadcasts to the full (B, R, H) shape.

Distribution: pure data parallel over batch (64 items -> 8 items/core on 8
NeuronCores), no collectives.  Per core the two HxH projections dominate:
8 * 2*(256+196)*1024*1024 ~= 7.6 GFLOP on the PE array in bf16.

Per-core engine plan:
  PE : projection matmuls (~96us of column streaming).  All 8 items are
       kept resident so each 128x128 weight chunk is loaded once per
       iteration and reused by 4 matmuls (pairs) -> 4x fewer LDWEIGHTS.
       The score dot is folded into a broadcast matmul (lhsT =
       outer(wv_oc, ones)) accumulating logits replicated across all 128
       partitions.
  ACT: tanh (bias per-partition) and exp (accum_out = softmax
       denominator).
  DVE: fused multiply+reduce (scalar_tensor_tensor) of the transposed
       activations against the exp row -> unnormalized attention sums,
       then a per-partition scalar multiply by 1/Z.
  DMA: bf16 transposed activations pre-packed on host into the exact
       SBUF tile layout (one 128x8KB descriptor set per pair), bf16
       weights, 128x8 f32 output tile per item per side.
"""

import os
import sys

import numpy as np

try:
    import concourse  # noqa: F401
except ImportError:  # fresh environment: fall back to the repo path
    sys.path.insert(0, "/opt/trn_rl_repo")

B, S, R, H = 64, 256, 196, 1024
NCORES = 8
BPC = B // NCORES        # batch items per core = 8
PAIRS = BPC // 2         # pairs of items = 4
HC = H // 128            # 8 chunks of the feature dim
ND = 2 * S               # dns pair free width  = 512
NG = 2 * R               # img pair free width  = 392

_CACHE = {}


def _build_program(loop_n=0):
    import concourse.bacc as bacc
    import concourse.tile as tile
    from concourse import mybir
    from contextlib import ExitStack
    import contextlib

    f32 = mybir.dt.float32
    bf16 = mybir.dt.bfloat16
    Act = mybir.ActivationFunctionType
    Alu = mybir.AluOpType

    nc = bacc.Bacc("TRN2", target_bir_lowering=False, debug=False)

    # activations pre-packed on host into the SBUF tile layout:
    # dtT[pr, p, hc*ND + j*S + s] = dns[2*pr + j, s, hc*128+p]
    dtT = nc.dram_tensor("dtT", (PAIRS, 128, HC * ND), bf16, kind="ExternalInput").ap()
    gtT = nc.dram_tensor("gtT", (PAIRS, 128, HC * NG), bf16, kind="ExternalInput").ap()
    w1t = nc.dram_tensor("w1t", (H, H), bf16, kind="ExternalInput").ap()
    w4t = nc.dram_tensor("w4t", (H, H), bf16, kind="ExternalInput").ap()
    bc1 = nc.dram_tensor("bc1", (128, HC), f32, kind="ExternalInput").ap()
    bc4 = nc.dram_tensor("bc4", (128, HC), f32, kind="ExternalInput").ap()
    wb1 = nc.dram_tensor("wb1", (128, HC * 128), bf16, kind="ExternalInput").ap()
    wb4 = nc.dram_tensor("wb4", (128, HC * 128), bf16, kind="ExternalInput").ap()

    o_dns = nc.dram_tensor("o_dns", (BPC, 128, HC), f32, kind="ExternalOutput").ap()
    o_img = nc.dram_tensor("o_img", (BPC, 128, HC), f32, kind="ExternalOutput").ap()

    with tile.TileContext(nc) as tc, ExitStack() as ctx:
        consts = ctx.enter_context(tc.tile_pool(name="consts", bufs=1))
        acts = ctx.enter_context(tc.tile_pool(name="acts", bufs=2 * PAIRS))
        tpool = ctx.enter_context(tc.tile_pool(name="tpool", bufs=2 * PAIRS))
        epool = ctx.enter_context(tc.tile_pool(name="epool", bufs=4))
        scrp = ctx.enter_context(tc.tile_pool(name="scrp", bufs=2))
        smalls = ctx.enter_context(tc.tile_pool(name="smalls", bufs=4))
        pproj = ctx.enter_context(tc.tile_pool(name="pproj", bufs=PAIRS, space="PSUM"))
        pbc = ctx.enter_context(tc.tile_pool(name="pbc", bufs=PAIRS, space="PSUM"))

        # --- constants ---
        w1_sb = consts.tile([128, HC * H], bf16, name="w1_sb")
        nc.sync.dma_start(
            out=w1_sb.rearrange("p (hc o) -> p hc o", hc=HC),
            in_=w1t.rearrange("(hc p) o -> p hc o", p=128),
        )
        w4_sb = consts.tile([128, HC * H], bf16, name="w4_sb")
        nc.sync.dma_start(
            out=w4_sb.rearrange("p (hc o) -> p hc o", hc=HC),
            in_=w4t.rearrange("(hc p) o -> p hc o", p=128),
        )
        b1_sb = consts.tile([128, HC], f32, name="b1_sb")
        nc.sync.dma_start(out=b1_sb, in_=bc1)
        b4_sb = consts.tile([128, HC], f32, name="b4_sb")
        nc.sync.dma_start(out=b4_sb, in_=bc4)
        wb1_sb = consts.tile([128, HC * 128], bf16, name="wb1_sb")
        nc.sync.dma_start(out=wb1_sb, in_=wb1)
        wb4_sb = consts.tile([128, HC * 128], bf16, name="wb4_sb")
        nc.sync.dma_start(out=wb4_sb, in_=wb4)

        loop_cm = (tc.For_i(0, loop_n, 1, hint_engines=(mybir.EngineType.PE,))
                   if loop_n else contextlib.nullcontext())
        with loop_cm:
            dts, gts = [], []
            for pr in range(PAIRS):
                dt = acts.tile([128, HC * ND], bf16, tag="dt", name=f"dt{pr}")
                gt = acts.tile([128, HC * NG], bf16, tag="gt", name=f"gt{pr}")
                nc.sync.dma_start(out=dt, in_=dtT[pr])
                nc.sync.dma_start(out=gt, in_=gtT[pr])
                dts.append(dt)
                gts.append(gt)

            for side in (0, 1):
                if side == 0:
                    act_ts, w_sb, b_sb, wb_sb, n, ns, out_ap = (
                        dts, w1_sb, b1_sb, wb1_sb, ND, S, o_dns)
                else:
                    act_ts, w_sb, b_sb, wb_sb, n, ns, out_ap = (
                        gts, w4_sb, b4_sb, wb4_sb, NG, R, o_img)

                # bc[pr][p, s] = sum_o wv[o] * tanh(proj[o, s] + b[o])  (same
                # for every partition p: lhsT columns of wb_sb are identical)
                bcs = [pbc.tile([128, n], f32, tag="bc", name=f"bc{side}_{pr}")
                       for pr in range(PAIRS)]
                for oc in range(HC):
                    pjs = [pproj.tile([128, n], f32, tag="proj",
                                      name=f"pj{side}_{oc}_{pr}")
                           for pr in range(PAIRS)]
                    # weight chunk (hc, oc) is stationary across the 4 pairs
                    for hc in range(HC):
                        for pr in range(PAIRS):
                            nc.tensor.matmul(
                                pjs[pr],
                                lhsT=w_sb[:, hc * H + oc * 128: hc * H + (oc + 1) * 128],
                                rhs=act_ts[pr][:, hc * n:(hc + 1) * n],
                                start=(hc == 0),
                                stop=(hc == HC - 1),
                            )
                    for pr in range(PAIRS):
                        tt = tpool.tile([128, n], bf16, tag="T",
                                        name=f"tt{side}_{oc}_{pr}")
                        nc.scalar.activation(
                            out=tt, in_=pjs[pr], func=Act.Tanh,
                            bias=b_sb[:, oc:oc + 1], scale=1.0,
                        )
                        nc.tensor.matmul(
                            bcs[pr],
                            lhsT=wb_sb[:, oc * 128:(oc + 1) * 128],
                            rhs=tt,
                            start=(oc == 0),
                            stop=(oc == HC - 1),
                        )

                # softmax over each item's slice of bc.  The logits are
                # bounded (|s| <= sum|w| ~ 16) so max-subtraction is
                # unnecessary in fp32.
                for pr in range(PAIRS):
                    for j in (0, 1):
                        it = 2 * pr + j
                        e = epool.tile([128, ns], bf16, tag="e", name=f"e{side}_{it}")
                        zs = smalls.tile([128, 1], f32, tag="zs", name=f"zs{side}_{it}")
                        nc.scalar.activation(
                            out=e, in_=bcs[pr][:, j * ns:(j + 1) * ns],
                            func=Act.Exp, accum_out=zs,
                        )
                        rz = smalls.tile([128, 1], f32, tag="rz", name=f"rz{side}_{it}")
                        nc.vector.reciprocal(out=rz, in_=zs)

                        # u[p, hc] = sum_s X.T[hc*128+p, s] * e[s]  via fused
                        # multiply+reduce on the vector engine
                        u = smalls.tile([128, HC], f32, tag="u", name=f"u{side}_{it}")
                        scr = scrp.tile([128, ns], bf16, tag="scr",
                                        name=f"scr{side}_{it}")
                        for hc in range(HC):
                            nc.vector.scalar_tensor_tensor(
                                out=scr,
                                in0=act_ts[pr][:, hc * n + j * ns: hc * n + j * ns + ns],
                                scalar=1.0,
                                in1=e,
                                op0=Alu.mult,
                                op1=Alu.mult,
                                accum_out=u[:, hc:hc + 1],
                            )
                        v = smalls.tile([128, HC], f32, tag="v", name=f"v{side}_{it}")
                        nc.vector.tensor_scalar_mul(v, u, rz)
                        nc.scalar.dma_start(out=out_ap[it], in_=v)

    nc.compile()
    return nc


def _get_program(loop_n=0):
    key = ("prog", loop_n)
    if key not in _CACHE:
        _CACHE[key] = _build_program(loop_n=loop_n)
    return _CACHE[key]


def _pack_pairs(xt):
    """(BPC, H, n) transposed activations -> (PAIRS, 128, HC*2*n) tile pack."""
    n = xt.shape[2]
    # (pr, j, hc, p, s) -> (pr, p, hc, j, s)
    arr = xt.reshape(PAIRS, 2, HC, 128, n).transpose(0, 3, 2, 1, 4)
    return np.ascontiguousarray(arr.reshape(PAIRS, 128, HC * 2 * n))


def _prepare_in_maps(dns_feature, img_features, W_dns1, b_dns1, W_img2, b_img2,
                     w_att1, w_att2):
    import ml_dtypes
    bf16 = ml_dtypes.bfloat16

    dns = np.asarray(dns_feature, np.float32).transpose(0, 2, 1).astype(bf16)
    img = np.asarray(img_features, np.float32).transpose(0, 2, 1).astype(bf16)
    w1t = np.ascontiguousarray(np.asarray(W_dns1, np.float32).T.astype(bf16))
    w4t = np.ascontiguousarray(np.asarray(W_img2, np.float32).T.astype(bf16))
    bc1 = np.ascontiguousarray(np.asarray(b_dns1, np.float32).reshape(HC, 128).T)
    bc4 = np.ascontiguousarray(np.asarray(b_img2, np.float32).reshape(HC, 128).T)
    # wb*[p, oc*128 + q] = wv[oc*128 + p]  (broadcast along q)
    wv1 = np.asarray(w_att1, np.float32)[H:].reshape(HC, 128).T  # (128, HC)
    wv2 = np.asarray(w_att2, np.float32)[H:].reshape(HC, 128).T
    wb1 = np.ascontiguousarray(
        np.repeat(wv1[:, :, None], 128, axis=2).reshape(128, HC * 128).astype(bf16))
    wb4 = np.ascontiguousarray(
        np.repeat(wv2[:, :, None], 128, axis=2).reshape(128, HC * 128).astype(bf16))
    in_maps = []
    for c in range(NCORES):
        in_maps.append({
            "dtT": _pack_pairs(dns[c * BPC:(c + 1) * BPC]),
            "gtT": _pack_pairs(img[c * BPC:(c + 1) * BPC]),
            "w1t": w1t, "w4t": w4t, "bc1": bc1, "bc4": bc4,
            "wb1": wb1, "wb4": wb4,
        })
    return in_maps


def _rows_to_full(rows, reps):
    """(B, 128, HC) device tile -> full (B, reps, H) output."""
    flat = rows.transpose(0, 2, 1).reshape(B, H)  # h = hc*128 + p
    return np.ascontiguousarray(
        np.broadcast_to(flat[:, None, :], (B, reps, H)))


def run(inputs, trace=False):
    """Run on the 8 NeuronCores; returns (att_img, att_dns, exec_time_ns)."""
    from concourse.bass_utils import run_bass_kernel_spmd

    nc = _get_program()
    in_maps = _prepare_in_maps(
        inputs["dns_feature"], inputs["img_features"],
        inputs["W_dns1"], inputs["b_dns1"], inputs["W_img2"], inputs["b_img2"],
        inputs["w_att1"], inputs["w_att2"],
    )
    res = run_bass_kernel_spmd(nc, in_maps, core_ids=list(range(NCORES)),
                               trace=trace)
    rows_dns = np.concatenate([res.results[c]["o_dns"] for c in range(NCORES)], 0)
    rows_img = np.concatenate([res.results[c]["o_img"] for c in range(NCORES)], 0)
    att_dns = _rows_to_full(np.asarray(rows_dns, np.float32), R)
    att_img = _rows_to_full(np.asarray(rows_img, np.float32), R)
    return att_img, att_dns, res.exec_time_ns


def kernel(**inputs):
    att_img, att_dns, _ = run(inputs, trace=False)
    return att_img, att_dns


if __name__ == "__main__":
    prog = _get_program()
    print("program built + compiled OK")


# revision 21
# speedup vs baseline: 1.8663x; 1.8663x over previous
"""Trainium2 Bass kernel for the CoAttention DNS/Image module.

Math notes (exact algebraic simplification of the reference):
  scores1[b,r,s] = s_img[b,r] + s_dns[b,s] + b_att1 ; softmax over s.
  The per-row constant s_img[b,r] (and b_att1) cancels in the softmax, so
  a1[b,r,:] == softmax(s_dns[b,:]) for every r. Hence
      att_dns[b,r,:] = softmax(s_dns[b]) @ dns[b]          (same for all r)
  Similarly scores2's softmax over j kills t_dns[b,i] and b_att2, so
      att_img[b,i,:] = softmax(t_img[b]) @ img[b]          (same for all i)
  Therefore W_img1, w_att1[:H], b_att1, W_dns2, w_att2[:H], b_att2 do not
  affect the outputs at all.  The remaining work per batch item:
      s_dns[s] = tanh(dns[b] @ W_dns1.T + b_dns1) @ w_att1[H:]
      t_img[j] = tanh(img[b] @ W_img2.T + b_img2) @ w_att2[H:]
  plus two softmaxes and two weighted sums.  Because every output row of
  att_dns[b] (and att_img[b]) is identical, the device only produces one
  row per item; the host broadcasts to the full (B, R, H) shape.

Distribution: pure data parallel over batch (64 items -> 8 items/core on 8
NeuronCores), no collectives.  Per core the two HxH projections dominate:
8 * 2*(256+196)*1024*1024 ~= 7.6 GFLOP on the PE array in bf16.

Per-core engine plan:
  PE : projection matmuls (~96us of column streaming).  All 8 items are
       kept resident so each 128x128 weight chunk is loaded once per
       iteration and reused by 4 matmuls (pairs) -> 4x fewer LDWEIGHTS.
       The score dot is folded into a broadcast matmul (lhsT =
       outer(wv_oc, ones)) accumulating logits replicated across all 128
       partitions.
  ACT: tanh (bias per-partition) and exp (accum_out = softmax
       denominator).
  DVE: fused multiply+reduce (scalar_tensor_tensor) of the transposed
       activations against the exp row -> unnormalized attention sums,
       then a per-partition scalar multiply by 1/Z.
  DMA: bf16 transposed activations pre-packed on host into the exact
       SBUF tile layout (one 128x8KB descriptor set per pair), bf16
       weights, 128x8 f32 output tile per item per side.
"""

import os
import sys

import numpy as np

try:
    import concourse  # noqa: F401
except ImportError:  # fresh environment: fall back to the repo path
    sys.path.insert(0, "/opt/trn_rl_repo")

B, S, R, H = 64, 256, 196, 1024
NCORES = 8
BPC = B // NCORES        # batch items per core = 8
PAIRS = BPC // 2         # pairs of items = 4
HC = H // 128            # 8 chunks of the feature dim
ND = 2 * S               # dns pair free width  = 512
NG = 2 * R               # img pair free width  = 392

_CACHE = {}


def _build_program(loop_n=0):
    import concourse.bacc as bacc
    import concourse.tile as tile
    from concourse import mybir
    from contextlib import ExitStack
    import contextlib

    f32 = mybir.dt.float32
    bf16 = mybir.dt.bfloat16
    Act = mybir.ActivationFunctionType
    Alu = mybir.AluOpType

    nc = bacc.Bacc("TRN2", target_bir_lowering=False, debug=False)

    # activations pre-packed on host into the SBUF tile layout:
    # dtT[pr, p, hc*ND + j*S + s] = dns[2*pr + j, s, hc*128+p]
    dtT = nc.dram_tensor("dtT", (PAIRS, 128, HC * ND), bf16, kind="ExternalInput").ap()
    gtT = nc.dram_tensor("gtT", (PAIRS, 128, HC * NG), bf16, kind="ExternalInput").ap()
    w1t = nc.dram_tensor("w1t", (H, H), bf16, kind="ExternalInput").ap()
    w4t = nc.dram_tensor("w4t", (H, H), bf16, kind="ExternalInput").ap()
    bc1 = nc.dram_tensor("bc1", (128, HC), f32, kind="ExternalInput").ap()
    bc4 = nc.dram_tensor("bc4", (128, HC), f32, kind="ExternalInput").ap()
    wb1 = nc.dram_tensor("wb1", (128, HC * 128), bf16, kind="ExternalInput").ap()
    wb4 = nc.dram_tensor("wb4", (128, HC * 128), bf16, kind="ExternalInput").ap()

    o_dns = nc.dram_tensor("o_dns", (BPC, 128, HC), f32, kind="ExternalOutput").ap()
    o_img = nc.dram_tensor("o_img", (BPC, 128, HC), f32, kind="ExternalOutput").ap()

    with tile.TileContext(nc) as tc, ExitStack() as ctx:
        consts = ctx.enter_context(tc.tile_pool(name="consts", bufs=1))
        acts = ctx.enter_context(tc.tile_pool(name="acts", bufs=2 * PAIRS))
        tpool = ctx.enter_context(tc.tile_pool(name="tpool", bufs=2 * PAIRS))
        epool = ctx.enter_context(tc.tile_pool(name="epool", bufs=4))
        scrp = ctx.enter_context(tc.tile_pool(name="scrp", bufs=2))
        smalls = ctx.enter_context(tc.tile_pool(name="smalls", bufs=4))
        pproj = ctx.enter_context(tc.tile_pool(name="pproj", bufs=PAIRS, space="PSUM"))
        pbc = ctx.enter_context(tc.tile_pool(name="pbc", bufs=PAIRS, space="PSUM"))

        # --- constants ---
        w1_sb = consts.tile([128, HC * H], bf16, name="w1_sb")
        nc.sync.dma_start(
            out=w1_sb.rearrange("p (hc o) -> p hc o", hc=HC),
            in_=w1t.rearrange("(hc p) o -> p hc o", p=128),
        )
        w4_sb = consts.tile([128, HC * H], bf16, name="w4_sb")
        nc.sync.dma_start(
            out=w4_sb.rearrange("p (hc o) -> p hc o", hc=HC),
            in_=w4t.rearrange("(hc p) o -> p hc o", p=128),
        )
        b1_sb = consts.tile([128, HC], f32, name="b1_sb")
        nc.sync.dma_start(out=b1_sb, in_=bc1)
        b4_sb = consts.tile([128, HC], f32, name="b4_sb")
        nc.sync.dma_start(out=b4_sb, in_=bc4)
        wb1_sb = consts.tile([128, HC * 128], bf16, name="wb1_sb")
        nc.sync.dma_start(out=wb1_sb, in_=wb1)
        wb4_sb = consts.tile([128, HC * 128], bf16, name="wb4_sb")
        nc.sync.dma_start(out=wb4_sb, in_=wb4)

        loop_cm = (tc.For_i(0, loop_n, 1, hint_engines=(mybir.EngineType.PE,))
                   if loop_n else contextlib.nullcontext())
        with loop_cm:
            dts, gts = [], []
            for pr in range(PAIRS):
                dt = acts.tile([128, HC * ND], bf16, tag="dt", name=f"dt{pr}")
                gt = acts.tile([128, HC * NG], bf16, tag="gt", name=f"gt{pr}")
                # dt rides the SP HWDGE ring, gt the ACT ring: the two input
                # streams issue and transfer in parallel
                nc.sync.dma_start(out=dt, in_=dtT[pr])
                nc.scalar.dma_start(out=gt, in_=gtT[pr])
                dts.append(dt)
                gts.append(gt)

            for side in (0, 1):
                if side == 0:
                    act_ts, w_sb, b_sb, wb_sb, n, ns, out_ap = (
                        dts, w1_sb, b1_sb, wb1_sb, ND, S, o_dns)
                else:
                    act_ts, w_sb, b_sb, wb_sb, n, ns, out_ap = (
                        gts, w4_sb, b4_sb, wb4_sb, NG, R, o_img)

                # bc[pr][p, s] = sum_o wv[o] * tanh(proj[o, s] + b[o])  (same
                # for every partition p: lhsT columns of wb_sb are identical)
                bcs = [pbc.tile([128, n], f32, tag="bc", name=f"bc{side}_{pr}")
                       for pr in range(PAIRS)]

                def flush(oc, prs, pjh):
                    # tanh + logit matmuls for a finished half-group; emitted
                    # one half-group late so the ACT round-trip hides behind
                    # the next 16 projection matmuls
                    for pr in prs:
                        tt = tpool.tile([128, n], bf16, tag="T",
                                        name=f"tt{side}_{oc}_{pr}")
                        nc.scalar.activation(
                            out=tt, in_=pjh[pr], func=Act.Tanh,
                            bias=b_sb[:, oc:oc + 1], scale=1.0,
                        )
                        nc.tensor.matmul(
                            bcs[pr],
                            lhsT=wb_sb[:, oc * 128:(oc + 1) * 128],
                            rhs=tt,
                            start=(oc == 0),
                            stop=(oc == HC - 1),
                        )

                pending = None
                for oc in range(HC):
                    for half in (0, 1):
                        prs = (0, 1) if half == 0 else (2, 3)
                        pjh = {pr: pproj.tile([128, n], f32, tag="proj",
                                              name=f"pj{side}_{oc}_{pr}")
                               for pr in prs}
                        # weight chunk (hc, oc) stationary across the 2 pairs
                        for hc in range(HC):
                            for pr in prs:
                                nc.tensor.matmul(
                                    pjh[pr],
                                    lhsT=w_sb[:, hc * H + oc * 128: hc * H + (oc + 1) * 128],
                                    rhs=act_ts[pr][:, hc * n:(hc + 1) * n],
                                    start=(hc == 0),
                                    stop=(hc == HC - 1),
                                )
                        if pending is not None:
                            flush(*pending)
                        pending = (oc, prs, pjh)
                if pending is not None:
                    flush(*pending)

                # softmax over each item's slice of bc.  The logits are
                # bounded (|s| <= sum|w| ~ 16) so max-subtraction is
                # unnecessary in fp32.
                for pr in range(PAIRS):
                    for j in (0, 1):
                        it = 2 * pr + j
                        e = epool.tile([128, ns], bf16, tag="e", name=f"e{side}_{it}")
                        zs = smalls.tile([128, 1], f32, tag="zs", name=f"zs{side}_{it}")
                        nc.scalar.activation(
                            out=e, in_=bcs[pr][:, j * ns:(j + 1) * ns],
                            func=Act.Exp, accum_out=zs,
                        )
                        rz = smalls.tile([128, 1], f32, tag="rz", name=f"rz{side}_{it}")
                        nc.vector.reciprocal(out=rz, in_=zs)

                        # u[p, hc] = sum_s X.T[hc*128+p, s] * e[s]  via fused
                        # multiply+reduce on the vector engine
                        u = smalls.tile([128, HC], f32, tag="u", name=f"u{side}_{it}")
                        scr = scrp.tile([128, ns], bf16, tag="scr",
                                        name=f"scr{side}_{it}")
                        for hc in range(HC):
                            nc.vector.scalar_tensor_tensor(
                                out=scr,
                                in0=act_ts[pr][:, hc * n + j * ns: hc * n + j * ns + ns],
                                scalar=1.0,
                                in1=e,
                                op0=Alu.mult,
                                op1=Alu.mult,
                                accum_out=u[:, hc:hc + 1],
                            )
                        v = smalls.tile([128, HC], f32, tag="v", name=f"v{side}_{it}")
                        nc.vector.tensor_scalar_mul(v, u, rz)
                        nc.scalar.dma_start(out=out_ap[it], in_=v)

    nc.compile()
    return nc


def _get_program(loop_n=0):
    key = ("prog", loop_n)
    if key not in _CACHE:
        _CACHE[key] = _build_program(loop_n=loop_n)
    return _CACHE[key]


def _pack_pairs(xt):
    """(BPC, H, n) transposed activations -> (PAIRS, 128, HC*2*n) tile pack."""
    n = xt.shape[2]
    # (pr, j, hc, p, s) -> (pr, p, hc, j, s)
    arr = xt.reshape(PAIRS, 2, HC, 128, n).transpose(0, 3, 2, 1, 4)
    return np.ascontiguousarray(arr.reshape(PAIRS, 128, HC * 2 * n))


def _prepare_in_maps(dns_feature, img_features, W_dns1, b_dns1, W_img2, b_img2,
                     w_att1, w_att2):
    import ml_dtypes
    bf16 = ml_dtypes.bfloat16

    dns = np.asarray(dns_feature, np.float32).transpose(0, 2, 1).astype(bf16)
    img = np.asarray(img_features, np.float32).transpose(0, 2, 1).astype(bf16)
    w1t = np.ascontiguousarray(np.asarray(W_dns1, np.float32).T.astype(bf16))
    w4t = np.ascontiguousarray(np.asarray(W_img2, np.float32).T.astype(bf16))
    bc1 = np.ascontiguousarray(np.asarray(b_dns1, np.float32).reshape(HC, 128).T)
    bc4 = np.ascontiguousarray(np.asarray(b_img2, np.float32).reshape(HC, 128).T)
    # wb*[p, oc*128 + q] = wv[oc*128 + p]  (broadcast along q)
    wv1 = np.asarray(w_att1, np.float32)[H:].reshape(HC, 128).T  # (128, HC)
    wv2 = np.asarray(w_att2, np.float32)[H:].reshape(HC, 128).T
    wb1 = np.ascontiguousarray(
        np.repeat(wv1[:, :, None], 128, axis=2).reshape(128, HC * 128).astype(bf16))
    wb4 = np.ascontiguousarray(
        np.repeat(wv2[:, :, None], 128, axis=2).reshape(128, HC * 128).astype(bf16))
    in_maps = []
    for c in range(NCORES):
        in_maps.append({
            "dtT": _pack_pairs(dns[c * BPC:(c + 1) * BPC]),
            "gtT": _pack_pairs(img[c * BPC:(c + 1) * BPC]),
            "w1t": w1t, "w4t": w4t, "bc1": bc1, "bc4": bc4,
            "wb1": wb1, "wb4": wb4,
        })
    return in_maps


def _rows_to_full(rows, reps):
    """(B, 128, HC) device tile -> full (B, reps, H) output."""
    flat = rows.transpose(0, 2, 1).reshape(B, H)  # h = hc*128 + p
    return np.ascontiguousarray(
        np.broadcast_to(flat[:, None, :], (B, reps, H)))


def run(inputs, trace=False):
    """Run on the 8 NeuronCores; returns (att_img, att_dns, exec_time_ns)."""
    from concourse.bass_utils import run_bass_kernel_spmd

    nc = _get_program()
    in_maps = _prepare_in_maps(
        inputs["dns_feature"], inputs["img_features"],
        inputs["W_dns1"], inputs["b_dns1"], inputs["W_img2"], inputs["b_img2"],
        inputs["w_att1"], inputs["w_att2"],
    )
    res = run_bass_kernel_spmd(nc, in_maps, core_ids=list(range(NCORES)),
                               trace=trace)
    rows_dns = np.concatenate([res.results[c]["o_dns"] for c in range(NCORES)], 0)
    rows_img = np.concatenate([res.results[c]["o_img"] for c in range(NCORES)], 0)
    att_dns = _rows_to_full(np.asarray(rows_dns, np.float32), R)
    att_img = _rows_to_full(np.asarray(rows_img, np.float32), R)
    return att_img, att_dns, res.exec_time_ns


def kernel(**inputs):
    att_img, att_dns, _ = run(inputs, trace=False)
    return att_img, att_dns


if __name__ == "__main__":
    prog = _get_program()
    print("program built + compiled OK")


# revision 22
# speedup vs baseline: 2.1336x; 1.1433x over previous
"""Trainium2 Bass kernel for the CoAttention DNS/Image module.

Math notes (exact algebraic simplification of the reference):
  scores1[b,r,s] = s_img[b,r] + s_dns[b,s] + b_att1 ; softmax over s.
  The per-row constant s_img[b,r] (and b_att1) cancels in the softmax, so
  a1[b,r,:] == softmax(s_dns[b,:]) for every r. Hence
      att_dns[b,r,:] = softmax(s_dns[b]) @ dns[b]          (same for all r)
  Similarly scores2's softmax over j kills t_dns[b,i] and b_att2, so
      att_img[b,i,:] = softmax(t_img[b]) @ img[b]          (same for all i)
  Therefore W_img1, w_att1[:H], b_att1, W_dns2, w_att2[:H], b_att2 do not
  affect the outputs at all.  The remaining work per batch item:
      s_dns[s] = tanh(dns[b] @ W_dns1.T + b_dns1) @ w_att1[H:]
      t_img[j] = tanh(img[b] @ W_img2.T + b_img2) @ w_att2[H:]
  plus two softmaxes and two weighted sums.  Because every output row of
  att_dns[b] (and att_img[b]) is identical, the device only produces one
  row per item; the host broadcasts to the full (B, R, H) shape.

Distribution: pure data parallel over batch (64 items -> 8 items/core on 8
NeuronCores), no collectives.  Per core the two HxH projections dominate:
8 * 2*(256+196)*1024*1024 ~= 7.6 GFLOP on the PE array in bf16.

Per-core engine plan:
  PE : projection matmuls (~96us of column streaming).  All 8 items are
       kept resident so each 128x128 weight chunk is loaded once per
       iteration and reused by 4 matmuls (pairs) -> 4x fewer LDWEIGHTS.
       The score dot is folded into a broadcast matmul (lhsT =
       outer(wv_oc, ones)) accumulating logits replicated across all 128
       partitions.
  ACT: tanh (bias per-partition) and exp (accum_out = softmax
       denominator).
  DVE: fused multiply+reduce (scalar_tensor_tensor) of the transposed
       activations against the exp row -> unnormalized attention sums,
       then a per-partition scalar multiply by 1/Z.
  DMA: bf16 transposed activations pre-packed on host into the exact
       SBUF tile layout (one 128x8KB descriptor set per pair), bf16
       weights, 128x8 f32 output tile per item per side.
"""

import os
import sys

import numpy as np

try:
    import concourse  # noqa: F401
except ImportError:  # fresh environment: fall back to the repo path
    sys.path.insert(0, "/opt/trn_rl_repo")

B, S, R, H = 64, 256, 196, 1024
NCORES = 8
BPC = B // NCORES        # batch items per core = 8
PAIRS = BPC // 2         # pairs of items = 4
HC = H // 128            # 8 chunks of the feature dim
ND = 2 * S               # dns pair free width  = 512
NG = 2 * R               # img pair free width  = 392

_CACHE = {}


def _build_program(loop_n=0):
    import concourse.bacc as bacc
    import concourse.tile as tile
    from concourse import mybir
    from contextlib import ExitStack
    import contextlib

    f32 = mybir.dt.float32
    bf16 = mybir.dt.bfloat16
    Act = mybir.ActivationFunctionType
    Alu = mybir.AluOpType

    nc = bacc.Bacc("TRN2", target_bir_lowering=False, debug=False)

    # activations pre-packed on host into the SBUF tile layout:
    # dtT[pr, p, hc*ND + j*S + s] = dns[2*pr + j, s, hc*128+p]
    dtT = nc.dram_tensor("dtT", (PAIRS, 128, HC * ND), bf16, kind="ExternalInput").ap()
    gtT = nc.dram_tensor("gtT", (PAIRS, 128, HC * NG), bf16, kind="ExternalInput").ap()
    w1t = nc.dram_tensor("w1t", (H, H), bf16, kind="ExternalInput").ap()
    w4t = nc.dram_tensor("w4t", (H, H), bf16, kind="ExternalInput").ap()
    bc1 = nc.dram_tensor("bc1", (128, HC), f32, kind="ExternalInput").ap()
    bc4 = nc.dram_tensor("bc4", (128, HC), f32, kind="ExternalInput").ap()
    wb1 = nc.dram_tensor("wb1", (128, HC * 128), bf16, kind="ExternalInput").ap()
    wb4 = nc.dram_tensor("wb4", (128, HC * 128), bf16, kind="ExternalInput").ap()

    o_dns = nc.dram_tensor("o_dns", (BPC, 128, HC), f32, kind="ExternalOutput").ap()
    o_img = nc.dram_tensor("o_img", (BPC, 128, HC), f32, kind="ExternalOutput").ap()

    with tile.TileContext(nc) as tc, ExitStack() as ctx:
        consts = ctx.enter_context(tc.tile_pool(name="consts", bufs=1))
        acts = ctx.enter_context(tc.tile_pool(name="acts", bufs=2 * PAIRS))
        tpool = ctx.enter_context(tc.tile_pool(name="tpool", bufs=2 * PAIRS))
        epool = ctx.enter_context(tc.tile_pool(name="epool", bufs=4))
        scrp = ctx.enter_context(tc.tile_pool(name="scrp", bufs=2))
        smalls = ctx.enter_context(tc.tile_pool(name="smalls", bufs=4))
        pproj = ctx.enter_context(tc.tile_pool(name="pproj", bufs=PAIRS, space="PSUM"))
        pbc = ctx.enter_context(tc.tile_pool(name="pbc", bufs=PAIRS, space="PSUM"))

        # --- constants ---
        w1_sb = consts.tile([128, HC * H], bf16, name="w1_sb")
        nc.sync.dma_start(
            out=w1_sb.rearrange("p (hc o) -> p hc o", hc=HC),
            in_=w1t.rearrange("(hc p) o -> p hc o", p=128),
        )
        w4_sb = consts.tile([128, HC * H], bf16, name="w4_sb")
        nc.sync.dma_start(
            out=w4_sb.rearrange("p (hc o) -> p hc o", hc=HC),
            in_=w4t.rearrange("(hc p) o -> p hc o", p=128),
        )
        b1_sb = consts.tile([128, HC], f32, name="b1_sb")
        nc.sync.dma_start(out=b1_sb, in_=bc1)
        b4_sb = consts.tile([128, HC], f32, name="b4_sb")
        nc.sync.dma_start(out=b4_sb, in_=bc4)
        wb1_sb = consts.tile([128, HC * 128], bf16, name="wb1_sb")
        nc.sync.dma_start(out=wb1_sb, in_=wb1)
        wb4_sb = consts.tile([128, HC * 128], bf16, name="wb4_sb")
        nc.sync.dma_start(out=wb4_sb, in_=wb4)

        loop_cm = (tc.For_i(0, loop_n, 1, hint_engines=(mybir.EngineType.PE,))
                   if loop_n else contextlib.nullcontext())
        with loop_cm:
            dts, gts = [], []
            for pr in range(PAIRS):
                dt = acts.tile([128, HC * ND], bf16, tag="dt", name=f"dt{pr}")
                gt = acts.tile([128, HC * NG], bf16, tag="gt", name=f"gt{pr}")
                nc.sync.dma_start(out=dt, in_=dtT[pr])
                nc.sync.dma_start(out=gt, in_=gtT[pr])
                dts.append(dt)
                gts.append(gt)

            for side in (0, 1):
                if side == 0:
                    act_ts, w_sb, b_sb, wb_sb, n, ns, out_ap = (
                        dts, w1_sb, b1_sb, wb1_sb, ND, S, o_dns)
                else:
                    act_ts, w_sb, b_sb, wb_sb, n, ns, out_ap = (
                        gts, w4_sb, b4_sb, wb4_sb, NG, R, o_img)

                # bc[pr][p, s] = sum_o wv[o] * tanh(proj[o, s] + b[o])  (same
                # for every partition p: lhsT columns of wb_sb are identical)
                bcs = [pbc.tile([128, n], f32, tag="bc", name=f"bc{side}_{pr}")
                       for pr in range(PAIRS)]
                for oc in range(HC):
                    pjs = [pproj.tile([128, n], f32, tag="proj",
                                      name=f"pj{side}_{oc}_{pr}")
                           for pr in range(PAIRS)]
                    # weight chunk (hc, oc) is stationary across the 4 pairs
                    for hc in range(HC):
                        for pr in range(PAIRS):
                            nc.tensor.matmul(
                                pjs[pr],
                                lhsT=w_sb[:, hc * H + oc * 128: hc * H + (oc + 1) * 128],
                                rhs=act_ts[pr][:, hc * n:(hc + 1) * n],
                                start=(hc == 0),
                                stop=(hc == HC - 1),
                            )
                    for pr in range(PAIRS):
                        tt = tpool.tile([128, n], bf16, tag="T",
                                        name=f"tt{side}_{oc}_{pr}")
                        nc.scalar.activation(
                            out=tt, in_=pjs[pr], func=Act.Tanh,
                            bias=b_sb[:, oc:oc + 1], scale=1.0,
                        )
                        nc.tensor.matmul(
                            bcs[pr],
                            lhsT=wb_sb[:, oc * 128:(oc + 1) * 128],
                            rhs=tt,
                            start=(oc == 0),
                            stop=(oc == HC - 1),
                        )

                # softmax over each item's slice of bc.  The logits are
                # bounded (|s| <= sum|w| ~ 16) so max-subtraction is
                # unnecessary in fp32.
                for pr in range(PAIRS):
                    for j in (0, 1):
                        it = 2 * pr + j
                        e = epool.tile([128, ns], bf16, tag="e", name=f"e{side}_{it}")
                        zs = smalls.tile([128, 1], f32, tag="zs", name=f"zs{side}_{it}")
                        nc.scalar.activation(
                            out=e, in_=bcs[pr][:, j * ns:(j + 1) * ns],
                            func=Act.Exp, accum_out=zs,
                        )
                        rz = smalls.tile([128, 1], f32, tag="rz", name=f"rz{side}_{it}")
                        nc.vector.reciprocal(out=rz, in_=zs)

                        # u[p, hc] = sum_s X.T[hc*128+p, s] * e[s]  via fused
                        # multiply+reduce on the vector engine
                        u = smalls.tile([128, HC], f32, tag="u", name=f"u{side}_{it}")
                        scr = scrp.tile([128, ns], bf16, tag="scr",
                                        name=f"scr{side}_{it}")
                        for hc in range(HC):
                            nc.vector.scalar_tensor_tensor(
                                out=scr,
                                in0=act_ts[pr][:, hc * n + j * ns: hc * n + j * ns + ns],
                                scalar=1.0,
                                in1=e,
                                op0=Alu.mult,
                                op1=Alu.mult,
                                accum_out=u[:, hc:hc + 1],
                            )
                        v = smalls.tile([128, HC], f32, tag="v", name=f"v{side}_{it}")
                        nc.vector.tensor_scalar_mul(v, u, rz)
                        nc.scalar.dma_start(out=out_ap[it], in_=v)

    nc.compile()
    return nc


def _get_program(loop_n=0):
    key = ("prog", loop_n)
    if key not in _CACHE:
        _CACHE[key] = _build_program(loop_n=loop_n)
    return _CACHE[key]


def _pack_pairs(xt):
    """(BPC, H, n) transposed activations -> (PAIRS, 128, HC*2*n) tile pack."""
    n = xt.shape[2]
    # (pr, j, hc, p, s) -> (pr, p, hc, j, s)
    arr = xt.reshape(PAIRS, 2, HC, 128, n).transpose(0, 3, 2, 1, 4)
    return np.ascontiguousarray(arr.reshape(PAIRS, 128, HC * 2 * n))


def _prepare_in_maps(dns_feature, img_features, W_dns1, b_dns1, W_img2, b_img2,
                     w_att1, w_att2):
    import ml_dtypes
    bf16 = ml_dtypes.bfloat16

    dns = np.asarray(dns_feature, np.float32).transpose(0, 2, 1).astype(bf16)
    img = np.asarray(img_features, np.float32).transpose(0, 2, 1).astype(bf16)
    w1t = np.ascontiguousarray(np.asarray(W_dns1, np.float32).T.astype(bf16))
    w4t = np.ascontiguousarray(np.asarray(W_img2, np.float32).T.astype(bf16))
    bc1 = np.ascontiguousarray(np.asarray(b_dns1, np.float32).reshape(HC, 128).T)
    bc4 = np.ascontiguousarray(np.asarray(b_img2, np.float32).reshape(HC, 128).T)
    # wb*[p, oc*128 + q] = wv[oc*128 + p]  (broadcast along q)
    wv1 = np.asarray(w_att1, np.float32)[H:].reshape(HC, 128).T  # (128, HC)
    wv2 = np.asarray(w_att2, np.float32)[H:].reshape(HC, 128).T
    wb1 = np.ascontiguousarray(
        np.repeat(wv1[:, :, None], 128, axis=2).reshape(128, HC * 128).astype(bf16))
    wb4 = np.ascontiguousarray(
        np.repeat(wv2[:, :, None], 128, axis=2).reshape(128, HC * 128).astype(bf16))
    in_maps = []
    for c in range(NCORES):
        in_maps.append({
            "dtT": _pack_pairs(dns[c * BPC:(c + 1) * BPC]),
            "gtT": _pack_pairs(img[c * BPC:(c + 1) * BPC]),
            "w1t": w1t, "w4t": w4t, "bc1": bc1, "bc4": bc4,
            "wb1": wb1, "wb4": wb4,
        })
    return in_maps


def _rows_to_full(rows, reps):
    """(B, 128, HC) device tile -> full (B, reps, H) output."""
    flat = rows.transpose(0, 2, 1).reshape(B, H)  # h = hc*128 + p
    return np.ascontiguousarray(
        np.broadcast_to(flat[:, None, :], (B, reps, H)))


def run(inputs, trace=False):
    """Run on the 8 NeuronCores; returns (att_img, att_dns, exec_time_ns)."""
    from concourse.bass_utils import run_bass_kernel_spmd

    nc = _get_program()
    in_maps = _prepare_in_maps(
        inputs["dns_feature"], inputs["img_features"],
        inputs["W_dns1"], inputs["b_dns1"], inputs["W_img2"], inputs["b_img2"],
        inputs["w_att1"], inputs["w_att2"],
    )
    res = run_bass_kernel_spmd(nc, in_maps, core_ids=list(range(NCORES)),
                               trace=trace)
    rows_dns = np.concatenate([res.results[c]["o_dns"] for c in range(NCORES)], 0)
    rows_img = np.concatenate([res.results[c]["o_img"] for c in range(NCORES)], 0)
    att_dns = _rows_to_full(np.asarray(rows_dns, np.float32), R)
    att_img = _rows_to_full(np.asarray(rows_img, np.float32), R)
    return att_img, att_dns, res.exec_time_ns


def kernel(**inputs):
    att_img, att_dns, _ = run(inputs, trace=False)
    return att_img, att_dns


if __name__ == "__main__":
    prog = _get_program()
    print("program built + compiled OK")
